# revision 36
# baseline (speedup 1.0000x reference)
"""TRN2 Bass kernel for channel cross-attention (XCA-style).

Math (per batch element b, matching the jax reference):
  qp = Wq q + bq ; kp = Wk k + bk           (1x1 convs, q/k: (192, 16384))
  qn = qp / max(||qp||_row, eps) ; kn likewise (L2 norm over the 16384 axis)
  A  = softmax_d(qn_c . kn_d * temp_h)       per head (6 heads x 32 ch)
  out = Wo (A (Wv v + bv)) + bo

Strategy (one batch element per core, 8 cores) — DMA-traffic-minimal:
  Pass 1 streams q,k ONCE as fp8(e4m3), pixel-major, with a ones channel
  prepended (aq = [1; q], 193 ch).  Raw grams are accumulated in PSUM with
  fp8 DoubleRow matmuls (2 pixels per partition, 2x rate):
      Gqq = aq aq^T,  Gqk = aq ak^T,  Gkk = ak ak^T     (193x193 each)
  Everything attention-related derives from these on-chip:
      row norms   ||qp_c||^2 = diag(Wq'^T Gqq Wq'),  Wq' = [bq; Wq^T]
      logits      P = Wq'^T Gqk Wk'
  (fp8 is plenty here: logits are tiny (~1e-2) and softmax deviations
  contribute <1% of the output, so gram noise is damped ~100x.)
  The v path stays bf16 end-to-end: softmax -> block-diag A -> fold
  W_comb^T = (Wo A Wv)^T and b_comb on-chip; pass 2 is a single conv
  out^T = [v;1]^T [Wc^T; bc], streamed per 128-pixel chunk (M=pixels),
  written to DRAM as bf16 pixel-major (host transposes back).

  DMA totals ~19 MB/core (fp8 q,k + bf16 v,out) vs ~51 MB for f32.
"""

import numpy as np
from contextlib import ExitStack

import concourse.bass as bass
import concourse.tile as tile
from concourse import bacc, mybir

DIM = 192
HEADS = 6
CH = 32
HW = 16384
B = 8
C1 = DIM + 1              # aug channels (ones row first)

VT = 4                    # v tiles
PXT = HW // VT            # 4096 pixels per v tile
GPT = PXT // 256          # groups (256 px) per v tile
C1P = 208                 # padded channel block (DoubleRow step must be %16)
W1 = 16 * C1P             # fp8 bytes per 16-pixel DRAM row
# pass-1 q/k tiles (pixels): big first, small last to cut the PE tail
T_SIZES = [2048, 4096, 4096, 4096, 2048]
T_OFFS = [sum(T_SIZES[:i]) for i in range(len(T_SIZES))]
NT = len(T_SIZES)

NG2 = HW // 256           # 64 output groups (256 px)
GPS = 4                   # groups per output staging tile (1024 px)
NST = NG2 // GPS          # 16 staging tiles / output DMAs

F32 = mybir.dt.float32
BF16 = mybir.dt.bfloat16
F8 = mybir.dt.float8e4
DR = mybir.MatmulPerfMode.DoubleRow
IDENT = mybir.ActivationFunctionType.Identity
SQRT = mybir.ActivationFunctionType.Sqrt
EXP = mybir.ActivationFunctionType.Exp

USE_DR = True             # fp8 DoubleRow (2x PE) for the pass-1 grams

_CACHE = {}


def _build():
    nc = bacc.Bacc("TRN2", target_bir_lowering=False, debug=False)

    # pixel-major fp8 aug inputs: row r holds pixels 16r..16r+15, each 193ch
    q_d = nc.declare_dram_parameter("q8", [HW // 16, W1], F8, isOutput=False)
    k_d = nc.declare_dram_parameter("k8", [HW // 16, W1], F8, isOutput=False)
    # v: channel-major bf16, pixels permuted [even|odd] per 256-block, plus
    # a ones row (193) so pass 2's bias rides the matmul
    v_d = nc.declare_dram_parameter("vb", [C1, HW], BF16, isOutput=False)
    # bf16 weights pack: cols [Wq' | Wk' | Wv | Wo^T | bv]
    wb_d = nc.declare_dram_parameter("wb", [C1, 900], BF16, isOutput=False)
    # f32 smalls: [bo(192) | pad]
    wf_d = nc.declare_dram_parameter("wf", [1, 200], F32, isOutput=False)
    # per-partition columns: [invtemp2_top | sgn_top | invtemp2_bot | sgn_bot]
    wc_d = nc.declare_dram_parameter("wtc", [128, 4], F32, isOutput=False)
    # out^T, pixel-major bf16: (tile, group, partition, 2x192)
    out_d = nc.declare_dram_parameter("out", [NST, GPS, 128, 384], BF16,
                                      isOutput=True)

    with tile.TileContext(nc) as tc, ExitStack() as ctx:
        wp = ctx.enter_context(tc.tile_pool(name="weights", bufs=1))
        pp = ctx.enter_context(tc.tile_pool(name="post", bufs=1))
        vpool = ctx.enter_context(tc.tile_pool(name="v_res", bufs=1))

        wb_t = wp.tile([128, 900], BF16, tag="wb_t")
        wb_b = wp.tile([65, 900], BF16, tag="wb_b")
        wf_t = wp.tile([1, 200], F32, tag="wf")
        wtc = wp.tile([128, 4], F32, tag="wtc")

        wq_t = wb_t[:, 0:192]          # Wq' rows 0:128   [128, 192]
        wq_b = wb_b[:, 0:192]          # Wq' rows 128:193 [65, 192]
        wk_t = wb_t[:, 192:384]
        wk_b = wb_b[:, 192:384]
        wv_t = wb_t[:, 384:576]        # Wv rows 0:128
        wv_b = wb_b[0:64, 384:576]     # Wv rows 128:192
        wo_t = wb_t[:, 576:768]        # Wo^T rows 0:128
        wo_b = wb_b[0:64, 576:768]
        bv_t = wb_t[:, 768:769]
        bv_b = wb_b[0:64, 768:769]
        bo_r = wf_t[0:1, 0:192]
        ident = wb_t[:, 772:900]       # I128 for PE transposes

        ones128 = wp.tile([128, 2], BF16, tag="ones128")
        nc.vector.memset(ones128[:], 1.0)
        ones65 = wp.tile([65, 2], BF16, tag="ones65")
        nc.vector.memset(ones65[:], 1.0)
        # act-table warms: force the Sqrt/Exp table loads into idle windows
        warm = wp.tile([1, 2], F32, tag="warm")
        nc.vector.memset(warm[:], 1.0)
        warm2 = wp.tile([1, 2], F32, tag="warm2")
        eps128 = wp.tile([128, 1], F32, tag="eps128")
        nc.vector.memset(eps128[:], 1e-20)
        ones1 = wp.tile([1, 2], F32, tag="ones1")
        nc.vector.memset(ones1[:], 1.0)
        bd_t = pp.tile([128, DIM], BF16, tag="bd_t")
        nc.vector.memset(bd_t[:], 0.0)
        bd_b = pp.tile([64, DIM], BF16, tag="bd_b")
        nc.vector.memset(bd_b[:], 0.0)

        v_tiles = []

        # ---------------- pass 1: fp8 raw grams ----------------
        with tc.tile_pool(name="acc_psum", bufs=1, space="PSUM") as accp:
            gqq_t = accp.tile([128, C1], F32, tag="gqq_t")
            gqk_t = accp.tile([128, C1], F32, tag="gqk_t")
            gkk_t = accp.tile([128, C1], F32, tag="gkk_t")
            gqq_b = accp.tile([65, 65], F32, tag="gqq_b")
            gqk_b = accp.tile([65, C1], F32, tag="gqk_b")
            gkk_b = accp.tile([65, 65], F32, tag="gkk_b")

            with tc.tile_pool(name="p1_in", bufs=1) as inp:
                for i in range(NT):
                    SZ = T_SIZES[i]
                    ppp_ = SZ // 128          # pixels per partition
                    wcols = ppp_ * C1P
                    ngrp = SZ // 256
                    r0 = T_OFFS[i] // 16
                    nr = SZ // 16
                    qt = inp.tile([128, wcols], F8, tag=f"qt{i}")
                    kt = inp.tile([128, wcols], F8, tag=f"kt{i}")
                    qsrc = q_d[r0:r0 + nr, :]
                    ksrc = k_d[r0:r0 + nr, :]
                    if nr > 128:
                        qsrc = qsrc.rearrange("(p q) w -> p (q w)", p=128)
                        ksrc = ksrc.rearrange("(p q) w -> p (q w)", p=128)
                    elif nr < 128:
                        qq = 128 // nr
                        qsrc = qsrc.rearrange("r (q w) -> (r q) w", q=qq)
                        ksrc = ksrc.rearrange("r (q w) -> (r q) w", q=qq)
                    nc.sync.dma_start(qt[:], qsrc)
                    nc.sync.dma_start(kt[:], ksrc)
                    if i == 0:
                        nc.sync.dma_start(wb_t[:], wb_d[0:128, :])
                        nc.sync.dma_start(wb_b[:], wb_d[128:C1, :])
                        nc.sync.dma_start(wf_t[:], wf_d[:])
                        nc.sync.dma_start(wtc[:], wc_d[:])
                        nc.scalar.sqrt(warm2[:], warm[:])
                    for g in range(ngrp):
                        first = (i == 0 and g == 0)
                        last = (i == NT - 1 and g == ngrp - 1)
                        qg = qt[:, g * 2 * C1P:(g + 1) * 2 * C1P]
                        kg = kt[:, g * 2 * C1P:(g + 1) * 2 * C1P]
                        if USE_DR:
                            qs = qg.rearrange("p (two c) -> p two c", two=2)
                            ks = kg.rearrange("p (two c) -> p two c", two=2)
                            qm = qs[:, :, 0:C1]
                            km = ks[:, :, 0:C1]
                            mm = [
                                (gqq_t, qs[:, :, 0:128], qm),
                                (gqk_t, qs[:, :, 0:128], km),
                                (gkk_t, ks[:, :, 0:128], km),
                                (gqq_b, qs[:, :, 128:C1], qs[:, :, 128:C1]),
                                (gqk_b, qs[:, :, 128:C1], km),
                                (gkk_b, ks[:, :, 128:C1], ks[:, :, 128:C1]),
                            ]
                            for out, lhsT, rhs in mm:
                                nc.tensor.matmul(out[:], lhsT, rhs,
                                                 start=first, stop=last,
                                                 perf_mode=DR)
                        else:
                            for j in range(2):
                                qs = qg[:, j * C1P:j * C1P + C1]
                                ks = kg[:, j * C1P:j * C1P + C1]
                                st = first and j == 0
                                sp = last and j == 1
                                mm = [
                                    (gqq_t, qs[:, 0:128], qs),
                                    (gqk_t, qs[:, 0:128], ks),
                                    (gkk_t, ks[:, 0:128], ks),
                                    (gqq_b, qs[:, 128:C1], qs[:, 128:C1]),
                                    (gqk_b, qs[:, 128:C1], ks),
                                    (gkk_b, ks[:, 128:C1], ks[:, 128:C1]),
                                ]
                                for out, lhsT, rhs in mm:
                                    nc.tensor.matmul(out[:], lhsT, rhs,
                                                     start=st, stop=sp)
            # v prefetch: all AFTER q/k in the DMA queue, so pass-1 PE
            # (paced by q/k) never waits and the DMA engine never idles
            for i in range(VT):
                vt = vpool.tile([128, PXT], BF16, tag=f"vt{i}")
                nc.sync.dma_start(vt[:], v_d[0:128, i * PXT:(i + 1) * PXT])
                vbt = vpool.tile([65, PXT], BF16, tag=f"vb{i}")
                nc.sync.dma_start(vbt[:], v_d[128:C1, i * PXT:(i + 1) * PXT])
                v_tiles.append((vt, vbt))

            # gram copies PSUM -> SBUF bf16 (split ACT / DVE, k-grams first:
            # the k-norm chain is the longest pole of the post phase).
            # Gkk/Gqq are symmetric: their bottom [65,193] = [top-right^T |
            # 65x65 corner], so pass 1 only computed the corner and the
            # top-right is transposed here on the PE.
            with tc.tile_pool(name="tr_psum", bufs=1, space="PSUM") as trp:
                gkk_st = pp.tile([128, C1], BF16, tag="gkk_st")
                nc.scalar.copy(gkk_st[:], gkk_t[:])
                tr_k = trp.tile([65, 128], BF16, tag="tr_k")
                nc.tensor.transpose(tr_k[:], gkk_st[:, 128:C1], ident)
                gkk_sb = pp.tile([65, C1], BF16, tag="gkk_sb")
                nc.vector.tensor_copy(gkk_sb[:, 128:C1], gkk_b[:])
                nc.vector.tensor_copy(gkk_sb[:, 0:128], tr_k[:])
                gqq_st = pp.tile([128, C1], BF16, tag="gqq_st")
                nc.scalar.copy(gqq_st[:], gqq_t[:])
                tr_q = trp.tile([65, 128], BF16, tag="tr_q")
                nc.tensor.transpose(tr_q[:], gqq_st[:, 128:C1], ident)
                gqq_sb = pp.tile([65, C1], BF16, tag="gqq_sb")
                nc.scalar.copy(gqq_sb[:, 128:C1], gqq_b[:])
                nc.scalar.copy(gqq_sb[:, 0:128], tr_q[:])
                gqk_st = pp.tile([128, C1], BF16, tag="gqk_st")
                nc.scalar.copy(gqk_st[:], gqk_t[:])
                gqk_sb = pp.tile([65, C1], BF16, tag="gqk_sb")
                nc.vector.tensor_copy(gqk_sb[:], gqk_b[:])
        # gram PSUM banks free from here

        # ---------------- norms: dq = diag(Wq'^T Gqq Wq') ----------------
        # (PSUM is 8 banks; post tiles are spread over three sequential pools)
        with tc.tile_pool(name="nrm_psum", bufs=1, space="PSUM") as npp:
            tq_t = npp.tile([128, DIM], F32, tag="tq_t")
            tq_b = npp.tile([65, DIM], F32, tag="tq_b")
            tk_t = npp.tile([128, DIM], F32, tag="tk_t")
            tk_b = npp.tile([65, DIM], F32, tag="tk_b")
            # Tk first (k-norm chain gates P via wks), then Tq
            nc.tensor.matmul(tk_t[:], gkk_st[:, 0:128], wk_t,
                             start=True, stop=False)
            nc.tensor.matmul(tk_t[:], gkk_sb[:, 0:128], wk_b,
                             start=False, stop=True)
            nc.tensor.matmul(tk_b[:], gkk_st[:, 128:C1], wk_t,
                             start=True, stop=False)
            nc.tensor.matmul(tk_b[:], gkk_sb[:, 128:C1], wk_b,
                             start=False, stop=True)
            nc.tensor.matmul(tq_t[:], gqq_st[:, 0:128], wq_t,
                             start=True, stop=False)
            nc.tensor.matmul(tq_t[:], gqq_sb[:, 0:128], wq_b,
                             start=False, stop=True)
            nc.tensor.matmul(tq_b[:], gqq_st[:, 128:C1], wq_t,
                             start=True, stop=False)
            nc.tensor.matmul(tq_b[:], gqq_sb[:, 128:C1], wq_b,
                             start=False, stop=True)

            # pk/pq = W .* T (bf16); then d = column sums via ones matmul
            pk_t = pp.tile([128, DIM], BF16, tag="pk_t")
            nc.vector.tensor_mul(pk_t[:], wk_t, tk_t[:])
            pk_b = pp.tile([65, DIM], BF16, tag="pk_b")
            nc.vector.tensor_mul(pk_b[:], wk_b, tk_b[:])
            pq_t = pp.tile([128, DIM], BF16, tag="pq_t")
            nc.vector.tensor_mul(pq_t[:], wq_t, tq_t[:])
            pq_b = pp.tile([65, DIM], BF16, tag="pq_b")
            nc.vector.tensor_mul(pq_b[:], wq_b, tq_b[:])

            # ST = Gqk^T-contraction step: ST[j, c] = sum_i Gqk[i,j] Wq'[i,c]
            st_t = npp.tile([128, DIM], F32, tag="st_t")
            st_b = npp.tile([65, DIM], F32, tag="st_b")
            nc.tensor.matmul(st_t[:], gqk_st[:, 0:128], wq_t,
                             start=True, stop=False)
            nc.tensor.matmul(st_t[:], gqk_sb[:, 0:128], wq_b,
                             start=False, stop=True)
            nc.tensor.matmul(st_b[:], gqk_st[:, 128:C1], wq_t,
                             start=True, stop=False)
            nc.tensor.matmul(st_b[:], gqk_sb[:, 128:C1], wq_b,
                             start=False, stop=True)

            st_st = pp.tile([128, DIM], BF16, tag="st_st")
            nc.scalar.copy(st_st[:], st_t[:])
            st_sb = pp.tile([65, DIM], BF16, tag="st_sb")
            nc.vector.tensor_copy(st_sb[:], st_b[:])

        with tc.tile_pool(name="dqp_psum", bufs=1, space="PSUM") as dpp:
            # dq as a COLUMN (per-partition scale for the head extraction):
            # dq[c, 0] = sum_i pq[i, c] via rhs=ones
            # dk as a ROW (free-dim scale): dk[0, d] = sum_i pk[i, d]
            dk_r = dpp.tile([2, DIM], F32, tag="dk_r")
            nc.tensor.matmul(dk_r[:], ones128[:], pk_t[:],
                             start=True, stop=False)
            nc.tensor.matmul(dk_r[:], ones65[:], pk_b[:],
                             start=False, stop=True)
            dq_t = dpp.tile([128, 2], F32, tag="dq_t")
            dq_b = dpp.tile([64, 2], F32, tag="dq_b")
            nc.tensor.matmul(dq_t[:, 0:2], pq_t[:, 0:128], ones128[:],
                             start=True, stop=False)
            nc.tensor.matmul(dq_t[:, 0:2], pq_b[:, 0:128], ones65[:],
                             start=False, stop=True)
            nc.tensor.matmul(dq_b[:, 0:2], pq_t[:, 128:DIM], ones128[:],
                             start=True, stop=False)
            nc.tensor.matmul(dq_b[:, 0:2], pq_b[:, 128:DIM], ones65[:],
                             start=False, stop=True)

            # 1/sqrt straight off PSUM (bias avoids 0; q side folds 1/temp^2)
            sk_r = pp.tile([1, DIM], F32, tag="sk_r")
            nc.scalar.activation(sk_r[:], dk_r[0:1, :], SQRT, bias=eps128[0:1, :])
            sq_t = pp.tile([128, 1], F32, tag="sq_t")
            nc.scalar.activation(sq_t[:], dq_t[:, 0:1], SQRT,
                                 bias=eps128[:], scale=wtc[:, 0:1])
            sq_b = pp.tile([64, 1], F32, tag="sq_b")
            nc.scalar.activation(sq_b[:], dq_b[:, 0:1], SQRT,
                                 bias=eps128[0:64, :], scale=wtc[0:64, 2:3])
            rk_r = pp.tile([1, DIM], F32, tag="rk_r")
            nc.vector.reciprocal(rk_r[:], sk_r[:])
            # hoist the Exp table load here (ACT idle while DVE works);
            # input aliases sk_r so it cannot be scheduled before the sqrts
            nc.scalar.activation(warm2[:], sk_r[0:1, 0:2], EXP)
            rq0_t = pp.tile([128, 1], F32, tag="rq0_t")
            nc.vector.reciprocal(rq0_t[:], sq_t[:])
            rq0_b = pp.tile([64, 1], F32, tag="rq0_b")
            nc.vector.reciprocal(rq0_b[:], sq_b[:])
            rq_t = pp.tile([128, 1], F32, tag="rq_t")
            nc.vector.tensor_mul(rq_t[:], rq0_t[:], wtc[:, 1:2])
            rq_b = pp.tile([64, 1], F32, tag="rq_b")
            nc.vector.tensor_mul(rq_b[:], rq0_b[:], wtc[0:64, 3:4])
            # fold rk into Wk' columns, then P comes out pre-scaled
            Bt = pp.tile([128, DIM], F32, tag="Bt")
            nc.gpsimd.partition_broadcast(Bt[:], rk_r[:])
            wks_t = pp.tile([128, DIM], BF16, tag="wks_t")
            nc.vector.tensor_mul(wks_t[:], wk_t, Bt[:])
            wks_b = pp.tile([65, DIM], BF16, tag="wks_b")
            nc.vector.tensor_mul(wks_b[:], wk_b, Bt[0:65, :])

            # P = Wq'^T Gqk (Wk'*rkt) : P[c, d] = sum_j ST[j, c] wks[j, d]
            p_t = dpp.tile([128, DIM], F32, tag="p_t")
            p_b = dpp.tile([64, DIM], F32, tag="p_b")
            nc.tensor.matmul(p_t[:], st_st[:, 0:128], wks_t[:],
                             start=True, stop=False)
            nc.tensor.matmul(p_t[:], st_sb[:, 0:128], wks_b[:],
                             start=False, stop=True)
            nc.tensor.matmul(p_b[:], st_st[:, 128:DIM], wks_t[:],
                             start=True, stop=False)
            nc.tensor.matmul(p_b[:], st_sb[:, 128:DIM], wks_b[:],
                             start=False, stop=True)

            # per-head softmax: E = exp(rq_c * P-block) straight off PSUM
            e1 = pp.tile([128, CH], F32, tag="e1")
            e2 = pp.tile([64, CH], F32, tag="e2")
            for h in range(4):
                hs = slice(h * CH, (h + 1) * CH)
                nc.scalar.activation(e1[hs, :], p_t[hs, hs], EXP,
                                     scale=rq_t[hs, 0:1])
            for h in range(4, HEADS):
                ps = slice((h - 4) * CH, (h - 3) * CH)
                hs = slice(h * CH, (h + 1) * CH)
                nc.scalar.activation(e2[ps, :], p_b[ps, hs], EXP,
                                     scale=rq_b[ps, 0:1])
            den1 = pp.tile([128, 1], F32, tag="den1")
            nc.vector.reduce_sum(den1[:], e1[:], axis=mybir.AxisListType.X)
            den2 = pp.tile([64, 1], F32, tag="den2")
            nc.vector.reduce_sum(den2[:], e2[:], axis=mybir.AxisListType.X)
            rden1 = pp.tile([128, 1], F32, tag="rden1")
            nc.vector.reciprocal(rden1[:], den1[:])
            rden2 = pp.tile([64, 1], F32, tag="rden2")
            nc.vector.reciprocal(rden2[:], den2[:])
            # fold 1/den into Wo^T rows (DVE, overlaps the ACT exps); the
            # block-diagonal A then carries raw exp(E) and Pool just copies
            wos_t = pp.tile([128, DIM], BF16, tag="wos_t")
            nc.vector.tensor_scalar_mul(wos_t[:], wo_t, rden1[:, 0:1])
            wos_b = pp.tile([64, DIM], BF16, tag="wos_b")
            nc.vector.tensor_scalar_mul(wos_b[:], wo_b, rden2[:, 0:1])

        with tc.tile_pool(name="p_psum", bufs=1, space="PSUM") as ppp:
            for h in range(4):
                hs = slice(h * CH, (h + 1) * CH)
                nc.gpsimd.tensor_copy(bd_t[hs, hs], e1[hs, :])
            for h in range(4, HEADS):
                ps = slice((h - 4) * CH, (h - 3) * CH)
                hs = slice(h * CH, (h + 1) * CH)
                nc.gpsimd.tensor_copy(bd_b[ps, hs], e2[ps, :])

            # ---- folds: X = (Wo A)^T ; Wc^T = Wv^T-contract X ; bc ----
            x_t = ppp.tile([128, DIM], F32, tag="x_t")
            x_b = ppp.tile([64, DIM], F32, tag="x_b")
            nc.tensor.matmul(x_t[:], bd_t[:, 0:128], wos_t[:],
                             start=True, stop=False)
            nc.tensor.matmul(x_t[:], bd_b[:, 0:128], wos_b[:],
                             start=False, stop=True)
            nc.tensor.matmul(x_b[:], bd_t[:, 128:DIM], wos_t[:],
                             start=True, stop=False)
            nc.tensor.matmul(x_b[:], bd_b[:, 128:DIM], wos_b[:],
                             start=False, stop=True)
            x_st = pp.tile([128, DIM], BF16, tag="x_st")
            nc.scalar.copy(x_st[:], x_t[:])
            x_sb = pp.tile([64, DIM], BF16, tag="x_sb")
            nc.vector.tensor_copy(x_sb[:], x_b[:])

            wc_t = ppp.tile([128, DIM], F32, tag="wc_t")
            wc_b = ppp.tile([64, DIM], F32, tag="wc_b")
            brow = ppp.tile([1, DIM], F32, tag="brow")
            nc.tensor.matmul(brow[:], bv_t, x_st[:], start=True, stop=False)
            nc.tensor.matmul(brow[:], bv_b, x_sb[:], start=False, stop=False)
            nc.tensor.matmul(brow[:], ones1[0:1, 0:1], bo_r, start=False, stop=True)
            nc.tensor.matmul(wc_t[:], wv_t[:, 0:128], x_st[:],
                             start=True, stop=False)
            nc.tensor.matmul(wc_t[:], wv_b[:, 0:128], x_sb[:],
                             start=False, stop=True)
            nc.tensor.matmul(wc_b[:], wv_t[:, 128:DIM], x_st[:],
                             start=True, stop=False)
            nc.tensor.matmul(wc_b[:], wv_b[:, 128:DIM], x_sb[:],
                             start=False, stop=True)

            # pass-2 rhs: [Wc^T; bc] split at v-channel 128 (+ones row)
            w2_t = pp.tile([128, DIM], BF16, tag="w2_t")
            nc.scalar.copy(w2_t[:], wc_t[:])
            w2_b = pp.tile([65, DIM], BF16, tag="w2_b")
            nc.vector.tensor_copy(w2_b[0:64, :], wc_b[:])
            nc.scalar.copy(w2_b[64:65, :], brow[:])

        # ---------------- pass 2: out^T = [v;1]^T [Wc^T; bc] ----------------
        with tc.tile_pool(name="p2_out", bufs=8) as op_, \
             tc.tile_pool(name="p2_psum", bufs=6, space="PSUM") as opp:
            st_plan = [(t, GPS) for t in range(NST - 1)] + [(NST - 1, 2), (NST - 1, 1), (NST - 1, 1)]
            gbase = 0
            for t, ngr in st_plan:
                ot = op_.tile([128, ngr * 384], BF16, tag="ot")
                for s in range(ngr):
                    g = gbase + s
                    vt, vbt = v_tiles[g // GPT]
                    off = (g % GPT) * 256  # GPT groups per v tile
                    pq2 = opp.tile([128, 2 * DIM], F32, tag="pq2")
                    pe = pq2[:, 0:DIM]
                    po = pq2[:, DIM:2 * DIM]
                    # one accumulation group spanning both halves of the bank:
                    # start zeroes the whole 2KB zero-region, later mms clear
                    # their own pending bytes on first write
                    nc.tensor.matmul(pe, vt[:, off:off + 128], w2_t[:],
                                     start=True, stop=False, skip_group_check=True)
                    nc.tensor.matmul(pe, vbt[:, off:off + 128], w2_b[:],
                                     start=False, stop=False, skip_group_check=True)
                    nc.tensor.matmul(po, vt[:, off + 128:off + 256], w2_t[:],
                                     start=False, stop=False, skip_group_check=True)
                    nc.tensor.matmul(po, vbt[:, off + 128:off + 256], w2_b[:],
                                     start=False, stop=True, skip_group_check=True)
                    od = ot[:, s * 384:(s + 1) * 384]
                    if s % 2 == 0:
                        nc.scalar.copy(od, pq2[:])
                    else:
                        nc.vector.tensor_copy(od, pq2[:])
                if ngr == GPS:
                    dst = out_d[t].transpose([1, 0, 2])
                else:
                    off4 = gbase - t * GPS
                    dst = out_d[t, off4:off4 + ngr].transpose([1, 0, 2])
                nc.sync.dma_start(dst, ot[:])
                gbase += ngr

    nc.compile()
    return nc


def _get_nc():
    if "nc" not in _CACHE:
        _CACHE["nc"] = _build()
    return _CACHE["nc"]


def _make_in_maps(inputs):
    import ml_dtypes
    f8 = ml_dtypes.float8_e4m3
    bf = ml_dtypes.bfloat16

    q = np.asarray(inputs["q"], dtype=np.float32)
    k = np.asarray(inputs["k"], dtype=np.float32)
    v = np.asarray(inputs["v"], dtype=np.float32)
    wq = np.asarray(inputs["wq"], dtype=np.float32)
    wk = np.asarray(inputs["wk"], dtype=np.float32)
    wv_ = np.asarray(inputs["wv"], dtype=np.float32)
    wo = np.asarray(inputs["wo"], dtype=np.float32)
    bq = np.asarray(inputs["bq"], dtype=np.float32)
    bk = np.asarray(inputs["bk"], dtype=np.float32)
    bv_ = np.asarray(inputs["bv"], dtype=np.float32)
    bo = np.asarray(inputs["bo"], dtype=np.float32)
    temp = np.asarray(inputs["temperature"], dtype=np.float32).reshape(HEADS)

    wb = np.zeros((C1, 900), dtype=np.float32)
    wb[0:128, 772:900] = np.eye(128, dtype=np.float32)
    wb[0, 0:192] = bq
    wb[1:, 0:192] = wq.T
    wb[0, 192:384] = bk
    wb[1:, 192:384] = wk.T
    wb[0:192, 384:576] = wv_
    wb[0:192, 576:768] = wo.T
    wb[0:192, 768] = bv_
    wf = np.zeros((1, 200), dtype=np.float32)
    wf[0, 0:192] = bo
    tcol = np.repeat(temp, CH)                  # (192,) per-channel temp
    wtc = np.zeros((128, 4), dtype=np.float32)
    wtc[:, 0] = 1.0 / np.maximum(tcol[0:128] ** 2, 1e-30)
    wtc[:, 1] = np.sign(tcol[0:128]) + (tcol[0:128] == 0)
    wtc[0:64, 2] = 1.0 / np.maximum(tcol[128:192] ** 2, 1e-30)
    wtc[0:64, 3] = np.sign(tcol[128:192]) + (tcol[128:192] == 0)

    shared = {
        "wb": np.ascontiguousarray(wb.astype(bf)),
        "wf": wf,
        "wtc": wtc,
    }
    ones_col = np.ones((HW, 1), dtype=np.float32)
    in_maps = []
    for b in range(B):
        m = dict(shared)
        aq = np.zeros((HW, C1P), dtype=np.float32)
        aq[:, 0:1] = 1.0
        aq[:, 1:C1] = q[b].reshape(DIM, HW).T
        ak = np.zeros((HW, C1P), dtype=np.float32)
        ak[:, 0:1] = 1.0
        ak[:, 1:C1] = k[b].reshape(DIM, HW).T
        m["q8"] = np.ascontiguousarray(
            aq.astype(f8).reshape(HW // 16, W1))
        m["k8"] = np.ascontiguousarray(
            ak.astype(f8).reshape(HW // 16, W1))
        # permute pixels [even|odd] per 256-block, append ones row
        vp = v[b].reshape(DIM, NG2, 128, 2).transpose(0, 1, 3, 2)
        vb = np.concatenate(
            [vp.reshape(DIM, HW), np.ones((1, HW), np.float32)], axis=0)
        m["vb"] = np.ascontiguousarray(vb.astype(bf))
        in_maps.append(m)
    return in_maps


def _get_runner():
    """Compile once and cache a sharded-jit runner."""
    if "runner" in _CACHE:
        return _CACHE["runner"]
    import jax
    import jax.numpy as jnp
    from jax.sharding import Mesh, PartitionSpec
    from jax.experimental.shard_map import shard_map
    from concourse import bass2jax, mybir as mb
    from concourse.bass2jax import _bass_exec_p, partition_id_tensor

    bass2jax.install_neuronx_cc_hook()
    nc = _get_nc()

    partition_name = nc.partition_id_tensor.name if nc.partition_id_tensor else None
    in_names, out_names, out_avals = [], [], []
    for alloc in nc.m.functions[0].allocations:
        if not isinstance(alloc, mb.MemoryLocationSet):
            continue
        name = alloc.memorylocations[0].name
        if alloc.kind == "ExternalInput":
            if name != partition_name:
                in_names.append(name)
        elif alloc.kind == "ExternalOutput":
            out_names.append(name)
            out_avals.append(jax.core.ShapedArray(
                tuple(alloc.tensor_shape), mb.dt.np(alloc.dtype)))
    n_params = len(in_names)
    n_outs = len(out_avals)
    all_in_names = tuple(in_names + out_names +
                         ([partition_name] if partition_name else []))

    def _body(*args):
        operands = list(args)
        if partition_name is not None:
            operands.append(partition_id_tensor())
        return tuple(_bass_exec_p.bind(
            *operands,
            out_avals=tuple(out_avals),
            in_names=all_in_names,
            out_names=tuple(out_names),
            lowering_input_output_aliases=(),
            sim_require_finite=True,
            sim_require_nnan=True,
            nc=nc,
        ))

    devices = jax.devices()[:B]
    mesh = Mesh(np.asarray(devices), ("core",))
    in_specs = (PartitionSpec("core"),) * (n_params + n_outs)
    out_specs = (PartitionSpec("core"),) * n_outs
    donate = tuple(range(n_params, n_params + n_outs))
    sharded = jax.jit(
        shard_map(_body, mesh=mesh, in_specs=in_specs, out_specs=out_specs,
                  check_rep=False),
        donate_argnums=donate, keep_unused=True)

    zero_shapes = [(B * a.shape[0], *a.shape[1:]) for a in out_avals]
    zero_dtypes = [a.dtype for a in out_avals]

    def run(in_maps):
        concat_in = [
            np.concatenate([np.asarray(in_maps[c][nm]) for c in range(B)], axis=0)
            for nm in in_names
        ]
        zeros = [jnp.zeros(s, d) for s, d in zip(zero_shapes, zero_dtypes)]
        outs = sharded(*concat_in, *zeros)
        return {
            nm: np.asarray(outs[i]).reshape(B, *out_avals[i].shape)
            for i, nm in enumerate(out_names)
        }

    _CACHE["runner"] = run
    return run


def _prebuild():
    """Compile the NEFF and warm the jit at import time; never break import.
    Uses non-degenerate dummy data so norms stay positive (rsqrt-safe)."""
    try:
        import ml_dtypes
        run = _get_runner()
        zq = np.ones((HW // 16, W1), dtype=ml_dtypes.float8_e4m3)
        zv = np.ones((C1, HW), dtype=ml_dtypes.bfloat16)
        zw = np.full((C1, 900), 0.01, dtype=ml_dtypes.bfloat16)
        zf = np.full((1, 200), 0.01, dtype=np.float32)
        zt = np.ones((128, 4), dtype=np.float32)
        run([{"q8": zq, "k8": zq, "vb": zv, "wb": zw, "wf": zf, "wtc": zt}
             for _ in range(B)])
    except Exception:
        _CACHE.clear()


def kernel(q, k, v, wq, bq, wk, bk, wv, bv, wo, bo, temperature):
    run = _get_runner()
    in_maps = _make_in_maps(dict(q=q, k=k, v=v, wq=wq, bq=bq, wk=wk, bk=bk,
                                 wv=wv, bv=bv, wo=wo, bo=bo,
                                 temperature=temperature))
    out = run(in_maps)["out"]  # (B, NST, GPS, 128, 384) bf16
    out = out.astype(np.float32).reshape(B, HW, DIM)
    out = np.ascontiguousarray(out.transpose(0, 2, 1)).reshape(B, DIM, 128, 128)
    return out


import os as _os
if not _os.environ.get("KERNEL_NO_PREBUILD"):
    _prebuild()


# revision 37
# speedup vs baseline: 1.0030x; 1.0030x over previous
"""TRN2 Bass kernel for channel cross-attention (XCA-style).

Math (per batch element b, matching the jax reference):
  qp = Wq q + bq ; kp = Wk k + bk           (1x1 convs, q/k: (192, 16384))
  qn = qp / max(||qp||_row, eps) ; kn likewise (L2 norm over the 16384 axis)
  A  = softmax_d(qn_c . kn_d * temp_h)       per head (6 heads x 32 ch)
  out = Wo (A (Wv v + bv)) + bo

Strategy (one batch element per core, 8 cores) — DMA-traffic-minimal:
  Pass 1 streams q,k ONCE as fp8(e4m3), pixel-major, with a ones channel
  prepended (aq = [1; q], 193 ch).  Raw grams are accumulated in PSUM with
  fp8 DoubleRow matmuls (2 pixels per partition, 2x rate):
      Gqq = aq aq^T,  Gqk = aq ak^T,  Gkk = ak ak^T     (193x193 each)
  Everything attention-related derives from these on-chip:
      row norms   ||qp_c||^2 = diag(Wq'^T Gqq Wq'),  Wq' = [bq; Wq^T]
      logits      P = Wq'^T Gqk Wk'
  (fp8 is plenty here: logits are tiny (~1e-2) and softmax deviations
  contribute <1% of the output, so gram noise is damped ~100x.)
  The v path stays bf16 end-to-end: softmax -> block-diag A -> fold
  W_comb^T = (Wo A Wv)^T and b_comb on-chip; pass 2 is a single conv
  out^T = [v;1]^T [Wc^T; bc], streamed per 128-pixel chunk (M=pixels),
  written to DRAM as bf16 pixel-major (host transposes back).

  DMA totals ~19 MB/core (fp8 q,k + bf16 v,out) vs ~51 MB for f32.
"""

import numpy as np
from contextlib import ExitStack

import concourse.bass as bass
import concourse.tile as tile
from concourse import bacc, mybir

DIM = 192
HEADS = 6
CH = 32
HW = 16384
B = 8
C1 = DIM + 1              # aug channels (ones row first)

VT = 4                    # v tiles
PXT = HW // VT            # 4096 pixels per v tile
GPT = PXT // 256          # groups (256 px) per v tile
C1P = 208                 # padded channel block (DoubleRow step must be %16)
W1 = 16 * C1P             # fp8 bytes per 16-pixel DRAM row
# pass-1 q/k tiles (pixels): big first, small last to cut the PE tail
T_SIZES = [2048, 4096, 4096, 4096, 2048]
T_OFFS = [sum(T_SIZES[:i]) for i in range(len(T_SIZES))]
NT = len(T_SIZES)

NG2 = HW // 256           # 64 output groups (256 px)
GPS = 4                   # groups per output staging tile (1024 px)
NST = NG2 // GPS          # 16 staging tiles / output DMAs

F32 = mybir.dt.float32
BF16 = mybir.dt.bfloat16
F8 = mybir.dt.float8e4
DR = mybir.MatmulPerfMode.DoubleRow
IDENT = mybir.ActivationFunctionType.Identity
SQRT = mybir.ActivationFunctionType.Sqrt
EXP = mybir.ActivationFunctionType.Exp

USE_DR = True             # fp8 DoubleRow (2x PE) for the pass-1 grams

_CACHE = {}


def _build():
    nc = bacc.Bacc("TRN2", target_bir_lowering=False, debug=False)

    # pixel-major fp8 aug inputs: row r holds pixels 16r..16r+15, each 193ch
    q_d = nc.declare_dram_parameter("q8", [HW // 16, W1], F8, isOutput=False)
    k_d = nc.declare_dram_parameter("k8", [HW // 16, W1], F8, isOutput=False)
    # v: channel-major bf16, pixels permuted [even|odd] per 256-block, plus
    # a ones row (193) so pass 2's bias rides the matmul
    v_d = nc.declare_dram_parameter("vb", [C1, HW], BF16, isOutput=False)
    # bf16 weights pack: cols [Wq' | Wk' | Wv | Wo^T | bv]
    wb_d = nc.declare_dram_parameter("wb", [C1, 900], BF16, isOutput=False)
    # f32 smalls: [bo(192) | pad]
    wf_d = nc.declare_dram_parameter("wf", [1, 200], F32, isOutput=False)
    # per-partition columns: [invtemp2_top | sgn_top | invtemp2_bot | sgn_bot]
    wc_d = nc.declare_dram_parameter("wtc", [128, 4], F32, isOutput=False)
    # out^T, pixel-major bf16: (tile, group, partition, 2x192)
    out_d = nc.declare_dram_parameter("out", [NST, GPS, 128, 384], BF16,
                                      isOutput=True)

    with tile.TileContext(nc) as tc, ExitStack() as ctx:
        wp = ctx.enter_context(tc.tile_pool(name="weights", bufs=1))
        pp = ctx.enter_context(tc.tile_pool(name="post", bufs=1))
        vpool = ctx.enter_context(tc.tile_pool(name="v_res", bufs=1))

        wb_t = wp.tile([128, 900], BF16, tag="wb_t")
        wb_b = wp.tile([65, 900], BF16, tag="wb_b")
        wf_t = wp.tile([1, 200], F32, tag="wf")
        wtc = wp.tile([128, 4], F32, tag="wtc")

        wq_t = wb_t[:, 0:192]          # Wq' rows 0:128   [128, 192]
        wq_b = wb_b[:, 0:192]          # Wq' rows 128:193 [65, 192]
        wk_t = wb_t[:, 192:384]
        wk_b = wb_b[:, 192:384]
        wv_t = wb_t[:, 384:576]        # Wv rows 0:128
        wv_b = wb_b[0:64, 384:576]     # Wv rows 128:192
        wo_t = wb_t[:, 576:768]        # Wo^T rows 0:128
        wo_b = wb_b[0:64, 576:768]
        bv_t = wb_t[:, 768:769]
        bv_b = wb_b[0:64, 768:769]
        bo_r = wf_t[0:1, 0:192]
        ident = wb_t[:, 772:900]       # I128 for PE transposes

        ones128 = wp.tile([128, 2], BF16, tag="ones128")
        nc.vector.memset(ones128[:], 1.0)
        ones65 = wp.tile([65, 2], BF16, tag="ones65")
        nc.vector.memset(ones65[:], 1.0)
        # act-table warms: force the Sqrt/Exp table loads into idle windows
        warm = wp.tile([1, 2], F32, tag="warm")
        nc.vector.memset(warm[:], 1.0)
        warm2 = wp.tile([1, 2], F32, tag="warm2")
        eps128 = wp.tile([128, 1], F32, tag="eps128")
        nc.vector.memset(eps128[:], 1e-20)
        ones1 = wp.tile([1, 2], F32, tag="ones1")
        nc.vector.memset(ones1[:], 1.0)
        bd_t = pp.tile([128, DIM], BF16, tag="bd_t")
        nc.vector.memset(bd_t[:], 0.0)
        bd_b = pp.tile([64, DIM], BF16, tag="bd_b")
        nc.vector.memset(bd_b[:], 0.0)

        v_tiles = []

        # ---------------- pass 1: fp8 raw grams ----------------
        with tc.tile_pool(name="acc_psum", bufs=1, space="PSUM") as accp:
            gqq_t = accp.tile([128, C1], F32, tag="gqq_t")
            gqk_t = accp.tile([128, C1], F32, tag="gqk_t")
            gkk_t = accp.tile([128, C1], F32, tag="gkk_t")
            gqq_b = accp.tile([65, 65], F32, tag="gqq_b")
            gqk_b = accp.tile([65, C1], F32, tag="gqk_b")
            gkk_b = accp.tile([65, 65], F32, tag="gkk_b")

            with tc.tile_pool(name="p1_in", bufs=1) as inp:
                for i in range(NT):
                    SZ = T_SIZES[i]
                    ppp_ = SZ // 128          # pixels per partition
                    wcols = ppp_ * C1P
                    ngrp = SZ // 256
                    r0 = T_OFFS[i] // 16
                    nr = SZ // 16
                    qt = inp.tile([128, wcols], F8, tag=f"qt{i}")
                    kt = inp.tile([128, wcols], F8, tag=f"kt{i}")
                    qsrc = q_d[r0:r0 + nr, :]
                    ksrc = k_d[r0:r0 + nr, :]
                    if nr > 128:
                        qsrc = qsrc.rearrange("(p q) w -> p (q w)", p=128)
                        ksrc = ksrc.rearrange("(p q) w -> p (q w)", p=128)
                    elif nr < 128:
                        qq = 128 // nr
                        qsrc = qsrc.rearrange("r (q w) -> (r q) w", q=qq)
                        ksrc = ksrc.rearrange("r (q w) -> (r q) w", q=qq)
                    nc.sync.dma_start(qt[:], qsrc)
                    nc.sync.dma_start(kt[:], ksrc)
                    if i == 0:
                        nc.sync.dma_start(wb_t[:], wb_d[0:128, :])
                        nc.sync.dma_start(wb_b[:], wb_d[128:C1, :])
                        nc.sync.dma_start(wf_t[:], wf_d[:])
                        nc.sync.dma_start(wtc[:], wc_d[:])
                        nc.scalar.sqrt(warm2[:], warm[:])
                    for g in range(ngrp):
                        first = (i == 0 and g == 0)
                        last = (i == NT - 1 and g == ngrp - 1)
                        qg = qt[:, g * 2 * C1P:(g + 1) * 2 * C1P]
                        kg = kt[:, g * 2 * C1P:(g + 1) * 2 * C1P]
                        if USE_DR:
                            qs = qg.rearrange("p (two c) -> p two c", two=2)
                            ks = kg.rearrange("p (two c) -> p two c", two=2)
                            qm = qs[:, :, 0:C1]
                            km = ks[:, :, 0:C1]
                            mm = [
                                (gqq_t, qs[:, :, 0:128], qm),
                                (gqk_t, qs[:, :, 0:128], km),
                                (gkk_t, ks[:, :, 0:128], km),
                                (gqq_b, qs[:, :, 128:C1], qs[:, :, 128:C1]),
                                (gqk_b, qs[:, :, 128:C1], km),
                                (gkk_b, ks[:, :, 128:C1], ks[:, :, 128:C1]),
                            ]
                            for out, lhsT, rhs in mm:
                                nc.tensor.matmul(out[:], lhsT, rhs,
                                                 start=first, stop=last,
                                                 perf_mode=DR)
                        else:
                            for j in range(2):
                                qs = qg[:, j * C1P:j * C1P + C1]
                                ks = kg[:, j * C1P:j * C1P + C1]
                                st = first and j == 0
                                sp = last and j == 1
                                mm = [
                                    (gqq_t, qs[:, 0:128], qs),
                                    (gqk_t, qs[:, 0:128], ks),
                                    (gkk_t, ks[:, 0:128], ks),
                                    (gqq_b, qs[:, 128:C1], qs[:, 128:C1]),
                                    (gqk_b, qs[:, 128:C1], ks),
                                    (gkk_b, ks[:, 128:C1], ks[:, 128:C1]),
                                ]
                                for out, lhsT, rhs in mm:
                                    nc.tensor.matmul(out[:], lhsT, rhs,
                                                     start=st, stop=sp)
            # v prefetch: all AFTER q/k in the DMA queue, so pass-1 PE
            # (paced by q/k) never waits and the DMA engine never idles
            for i in range(VT):
                vt = vpool.tile([128, PXT], BF16, tag=f"vt{i}")
                nc.sync.dma_start(vt[:], v_d[0:128, i * PXT:(i + 1) * PXT])
                vbt = vpool.tile([65, PXT], BF16, tag=f"vb{i}")
                nc.sync.dma_start(vbt[:], v_d[128:C1, i * PXT:(i + 1) * PXT])
                v_tiles.append((vt, vbt))

            # gram copies PSUM -> SBUF bf16 (split ACT / DVE, k-grams first:
            # the k-norm chain is the longest pole of the post phase).
            # Gkk/Gqq are symmetric: their bottom [65,193] = [top-right^T |
            # 65x65 corner], so pass 1 only computed the corner and the
            # top-right is transposed here on the PE.
            with tc.tile_pool(name="tr_psum", bufs=1, space="PSUM") as trp:
                gkk_st = pp.tile([128, C1], BF16, tag="gkk_st")
                nc.scalar.copy(gkk_st[:], gkk_t[:])
                tr_k = trp.tile([65, 128], BF16, tag="tr_k")
                nc.tensor.transpose(tr_k[:], gkk_st[:, 128:C1], ident)
                gkk_sb = pp.tile([65, C1], BF16, tag="gkk_sb")
                nc.vector.tensor_copy(gkk_sb[:, 128:C1], gkk_b[:])
                nc.vector.tensor_copy(gkk_sb[:, 0:128], tr_k[:])
                gqq_st = pp.tile([128, C1], BF16, tag="gqq_st")
                nc.scalar.copy(gqq_st[:], gqq_t[:])
                tr_q = trp.tile([65, 128], BF16, tag="tr_q")
                nc.tensor.transpose(tr_q[:], gqq_st[:, 128:C1], ident)
                gqq_sb = pp.tile([65, C1], BF16, tag="gqq_sb")
                nc.scalar.copy(gqq_sb[:, 128:C1], gqq_b[:])
                nc.scalar.copy(gqq_sb[:, 0:128], tr_q[:])
                gqk_st = pp.tile([128, C1], BF16, tag="gqk_st")
                nc.scalar.copy(gqk_st[:], gqk_t[:])
                gqk_sb = pp.tile([65, C1], BF16, tag="gqk_sb")
                nc.vector.tensor_copy(gqk_sb[:], gqk_b[:])
        # gram PSUM banks free from here

        # ---------------- norms: dq = diag(Wq'^T Gqq Wq') ----------------
        # (PSUM is 8 banks; post tiles are spread over three sequential pools)
        with tc.tile_pool(name="nrm_psum", bufs=1, space="PSUM") as npp:
            tq_t = npp.tile([128, DIM], F32, tag="tq_t")
            tq_b = npp.tile([65, DIM], F32, tag="tq_b")
            tk_t = npp.tile([128, DIM], F32, tag="tk_t")
            tk_b = npp.tile([65, DIM], F32, tag="tk_b")
            # Tk first (k-norm chain gates P via wks), then Tq
            nc.tensor.matmul(tk_t[:], gkk_st[:, 0:128], wk_t,
                             start=True, stop=False)
            nc.tensor.matmul(tk_t[:], gkk_sb[:, 0:128], wk_b,
                             start=False, stop=True)
            nc.tensor.matmul(tk_b[:], gkk_st[:, 128:C1], wk_t,
                             start=True, stop=False)
            nc.tensor.matmul(tk_b[:], gkk_sb[:, 128:C1], wk_b,
                             start=False, stop=True)
            nc.tensor.matmul(tq_t[:], gqq_st[:, 0:128], wq_t,
                             start=True, stop=False)
            nc.tensor.matmul(tq_t[:], gqq_sb[:, 0:128], wq_b,
                             start=False, stop=True)
            nc.tensor.matmul(tq_b[:], gqq_st[:, 128:C1], wq_t,
                             start=True, stop=False)
            nc.tensor.matmul(tq_b[:], gqq_sb[:, 128:C1], wq_b,
                             start=False, stop=True)

            # pk/pq = W .* T (bf16); then d = column sums via ones matmul
            pk_t = pp.tile([128, DIM], BF16, tag="pk_t")
            nc.vector.tensor_mul(pk_t[:], wk_t, tk_t[:])
            pk_b = pp.tile([65, DIM], BF16, tag="pk_b")
            nc.vector.tensor_mul(pk_b[:], wk_b, tk_b[:])
            pq_t = pp.tile([128, DIM], BF16, tag="pq_t")
            nc.vector.tensor_mul(pq_t[:], wq_t, tq_t[:])
            pq_b = pp.tile([65, DIM], BF16, tag="pq_b")
            nc.vector.tensor_mul(pq_b[:], wq_b, tq_b[:])

            # ST = Gqk^T-contraction step: ST[j, c] = sum_i Gqk[i,j] Wq'[i,c]
            st_t = npp.tile([128, DIM], F32, tag="st_t")
            st_b = npp.tile([65, DIM], F32, tag="st_b")
            nc.tensor.matmul(st_t[:], gqk_st[:, 0:128], wq_t,
                             start=True, stop=False)
            nc.tensor.matmul(st_t[:], gqk_sb[:, 0:128], wq_b,
                             start=False, stop=True)
            nc.tensor.matmul(st_b[:], gqk_st[:, 128:C1], wq_t,
                             start=True, stop=False)
            nc.tensor.matmul(st_b[:], gqk_sb[:, 128:C1], wq_b,
                             start=False, stop=True)

            st_st = pp.tile([128, DIM], BF16, tag="st_st")
            nc.scalar.copy(st_st[:], st_t[:])
            st_sb = pp.tile([65, DIM], BF16, tag="st_sb")
            nc.vector.tensor_copy(st_sb[:], st_b[:])

        with tc.tile_pool(name="dqp_psum", bufs=1, space="PSUM") as dpp:
            # dq as a COLUMN (per-partition scale for the head extraction):
            # dq[c, 0] = sum_i pq[i, c] via rhs=ones
            # dk as a ROW (free-dim scale): dk[0, d] = sum_i pk[i, d]
            dk_r = dpp.tile([2, DIM], F32, tag="dk_r")
            nc.tensor.matmul(dk_r[:], ones128[:], pk_t[:],
                             start=True, stop=False)
            nc.tensor.matmul(dk_r[:], ones65[:], pk_b[:],
                             start=False, stop=True)
            dq_t = dpp.tile([128, 2], F32, tag="dq_t")
            dq_b = dpp.tile([64, 2], F32, tag="dq_b")
            nc.tensor.matmul(dq_t[:, 0:2], pq_t[:, 0:128], ones128[:],
                             start=True, stop=False)
            nc.tensor.matmul(dq_t[:, 0:2], pq_b[:, 0:128], ones65[:],
                             start=False, stop=True)
            nc.tensor.matmul(dq_b[:, 0:2], pq_t[:, 128:DIM], ones128[:],
                             start=True, stop=False)
            nc.tensor.matmul(dq_b[:, 0:2], pq_b[:, 128:DIM], ones65[:],
                             start=False, stop=True)

            # 1/sqrt straight off PSUM (bias avoids 0; q side folds 1/temp^2)
            sk_r = pp.tile([1, DIM], F32, tag="sk_r")
            nc.scalar.activation(sk_r[:], dk_r[0:1, :], SQRT, bias=eps128[0:1, :])
            sq_t = pp.tile([128, 1], F32, tag="sq_t")
            nc.scalar.activation(sq_t[:], dq_t[:, 0:1], SQRT,
                                 bias=eps128[:], scale=wtc[:, 0:1])
            sq_b = pp.tile([64, 1], F32, tag="sq_b")
            nc.scalar.activation(sq_b[:], dq_b[:, 0:1], SQRT,
                                 bias=eps128[0:64, :], scale=wtc[0:64, 2:3])
            rk_r = pp.tile([1, DIM], F32, tag="rk_r")
            nc.vector.reciprocal(rk_r[:], sk_r[:])
            # hoist the Exp table load here (ACT idle while DVE works);
            # input aliases sk_r so it cannot be scheduled before the sqrts
            nc.scalar.activation(warm2[:], sk_r[0:1, 0:2], EXP)
            rq0_t = pp.tile([128, 1], F32, tag="rq0_t")
            nc.vector.reciprocal(rq0_t[:], sq_t[:])
            rq0_b = pp.tile([64, 1], F32, tag="rq0_b")
            nc.vector.reciprocal(rq0_b[:], sq_b[:])
            rq_t = pp.tile([128, 1], F32, tag="rq_t")
            nc.vector.tensor_mul(rq_t[:], rq0_t[:], wtc[:, 1:2])
            rq_b = pp.tile([64, 1], F32, tag="rq_b")
            nc.vector.tensor_mul(rq_b[:], rq0_b[:], wtc[0:64, 3:4])
            # fold rk into Wk' columns, then P comes out pre-scaled
            Bt = pp.tile([128, DIM], F32, tag="Bt")
            nc.gpsimd.partition_broadcast(Bt[:], rk_r[:])
            wks_t = pp.tile([128, DIM], BF16, tag="wks_t")
            nc.vector.tensor_mul(wks_t[:], wk_t, Bt[:])
            wks_b = pp.tile([65, DIM], BF16, tag="wks_b")
            nc.vector.tensor_mul(wks_b[:], wk_b, Bt[0:65, :])

            # P = Wq'^T Gqk (Wk'*rkt) : P[c, d] = sum_j ST[j, c] wks[j, d]
            p_t = dpp.tile([128, DIM], F32, tag="p_t")
            p_b = dpp.tile([64, DIM], F32, tag="p_b")
            nc.tensor.matmul(p_t[:], st_st[:, 0:128], wks_t[:],
                             start=True, stop=False)
            nc.tensor.matmul(p_t[:], st_sb[:, 0:128], wks_b[:],
                             start=False, stop=True)
            nc.tensor.matmul(p_b[:], st_st[:, 128:DIM], wks_t[:],
                             start=True, stop=False)
            nc.tensor.matmul(p_b[:], st_sb[:, 128:DIM], wks_b[:],
                             start=False, stop=True)

            # per-head softmax: E = exp(rq_c * P-block) straight off PSUM
            e1 = pp.tile([128, CH], F32, tag="e1")
            e2 = pp.tile([64, CH], F32, tag="e2")
            for h in range(4):
                hs = slice(h * CH, (h + 1) * CH)
                nc.scalar.activation(e1[hs, :], p_t[hs, hs], EXP,
                                     scale=rq_t[hs, 0:1])
            for h in range(4, HEADS):
                ps = slice((h - 4) * CH, (h - 3) * CH)
                hs = slice(h * CH, (h + 1) * CH)
                nc.scalar.activation(e2[ps, :], p_b[ps, hs], EXP,
                                     scale=rq_b[ps, 0:1])
            den1 = pp.tile([128, 1], F32, tag="den1")
            nc.vector.reduce_sum(den1[:], e1[:], axis=mybir.AxisListType.X)
            den2 = pp.tile([64, 1], F32, tag="den2")
            nc.vector.reduce_sum(den2[:], e2[:], axis=mybir.AxisListType.X)
            rden1 = pp.tile([128, 1], F32, tag="rden1")
            nc.vector.reciprocal(rden1[:], den1[:])
            rden2 = pp.tile([64, 1], F32, tag="rden2")
            nc.vector.reciprocal(rden2[:], den2[:])
            # fold 1/den into Wo^T rows (DVE, overlaps the ACT exps); the
            # block-diagonal A then carries raw exp(E) and Pool just copies
            wos_t = pp.tile([128, DIM], BF16, tag="wos_t")
            nc.vector.tensor_scalar_mul(wos_t[:], wo_t, rden1[:, 0:1])
            wos_b = pp.tile([64, DIM], BF16, tag="wos_b")
            nc.vector.tensor_scalar_mul(wos_b[:], wo_b, rden2[:, 0:1])

        with tc.tile_pool(name="p_psum", bufs=1, space="PSUM") as ppp:
            for h in range(4):
                hs = slice(h * CH, (h + 1) * CH)
                nc.gpsimd.tensor_copy(bd_t[hs, hs], e1[hs, :])
            for h in range(4, HEADS):
                ps = slice((h - 4) * CH, (h - 3) * CH)
                hs = slice(h * CH, (h + 1) * CH)
                nc.gpsimd.tensor_copy(bd_b[ps, hs], e2[ps, :])

            # ---- folds: X = (Wo A)^T ; Wc^T = Wv^T-contract X ; bc ----
            x_t = ppp.tile([128, DIM], F32, tag="x_t")
            x_b = ppp.tile([64, DIM], F32, tag="x_b")
            nc.tensor.matmul(x_t[:], bd_t[:, 0:128], wos_t[:],
                             start=True, stop=False)
            nc.tensor.matmul(x_t[:], bd_b[:, 0:128], wos_b[:],
                             start=False, stop=True)
            nc.tensor.matmul(x_b[:], bd_t[:, 128:DIM], wos_t[:],
                             start=True, stop=False)
            nc.tensor.matmul(x_b[:], bd_b[:, 128:DIM], wos_b[:],
                             start=False, stop=True)
            x_st = pp.tile([128, DIM], BF16, tag="x_st")
            nc.scalar.copy(x_st[:], x_t[:])
            x_sb = pp.tile([64, DIM], BF16, tag="x_sb")
            nc.vector.tensor_copy(x_sb[:], x_b[:])

            wc_t = ppp.tile([128, DIM], F32, tag="wc_t")
            wc_b = ppp.tile([64, DIM], F32, tag="wc_b")
            brow = ppp.tile([1, DIM], F32, tag="brow")
            nc.tensor.matmul(brow[:], bv_t, x_st[:], start=True, stop=False)
            nc.tensor.matmul(brow[:], bv_b, x_sb[:], start=False, stop=False)
            nc.tensor.matmul(brow[:], ones1[0:1, 0:1], bo_r, start=False, stop=True)
            nc.tensor.matmul(wc_t[:], wv_t[:, 0:128], x_st[:],
                             start=True, stop=False)
            nc.tensor.matmul(wc_t[:], wv_b[:, 0:128], x_sb[:],
                             start=False, stop=True)
            nc.tensor.matmul(wc_b[:], wv_t[:, 128:DIM], x_st[:],
                             start=True, stop=False)
            nc.tensor.matmul(wc_b[:], wv_b[:, 128:DIM], x_sb[:],
                             start=False, stop=True)

            # pass-2 rhs: [Wc^T; bc] split at v-channel 128 (+ones row)
            w2_t = pp.tile([128, DIM], BF16, tag="w2_t")
            nc.scalar.copy(w2_t[:], wc_t[:])
            w2_b = pp.tile([65, DIM], BF16, tag="w2_b")
            nc.vector.tensor_copy(w2_b[0:64, :], wc_b[:])
            nc.scalar.copy(w2_b[64:65, :], brow[:])

        # ---------------- pass 2: out^T = [v;1]^T [Wc^T; bc] ----------------
        with tc.tile_pool(name="p2_out", bufs=8) as op_, \
             tc.tile_pool(name="p2_psum", bufs=6, space="PSUM") as opp:
            st_plan = [(t, GPS) for t in range(NST - 1)] + [(NST - 1, 3), (NST - 1, 1)]
            gbase = 0
            for t, ngr in st_plan:
                ot = op_.tile([128, ngr * 384], BF16, tag="ot")
                for s in range(ngr):
                    g = gbase + s
                    vt, vbt = v_tiles[g // GPT]
                    off = (g % GPT) * 256  # GPT groups per v tile
                    pq2 = opp.tile([128, 2 * DIM], F32, tag="pq2")
                    pe = pq2[:, 0:DIM]
                    po = pq2[:, DIM:2 * DIM]
                    # one accumulation group spanning both halves of the bank:
                    # start zeroes the whole 2KB zero-region, later mms clear
                    # their own pending bytes on first write
                    nc.tensor.matmul(pe, vt[:, off:off + 128], w2_t[:],
                                     start=True, stop=False, skip_group_check=True)
                    nc.tensor.matmul(pe, vbt[:, off:off + 128], w2_b[:],
                                     start=False, stop=False, skip_group_check=True)
                    nc.tensor.matmul(po, vt[:, off + 128:off + 256], w2_t[:],
                                     start=False, stop=False, skip_group_check=True)
                    nc.tensor.matmul(po, vbt[:, off + 128:off + 256], w2_b[:],
                                     start=False, stop=True, skip_group_check=True)
                    od = ot[:, s * 384:(s + 1) * 384]
                    if s % 2 == 0:
                        nc.scalar.copy(od, pq2[:])
                    else:
                        nc.vector.tensor_copy(od, pq2[:])
                if ngr == GPS:
                    dst = out_d[t].transpose([1, 0, 2])
                else:
                    off4 = gbase - t * GPS
                    dst = out_d[t, off4:off4 + ngr].transpose([1, 0, 2])
                nc.sync.dma_start(dst, ot[:])
                gbase += ngr

    nc.compile()
    return nc


def _get_nc():
    if "nc" not in _CACHE:
        _CACHE["nc"] = _build()
    return _CACHE["nc"]


def _make_in_maps(inputs):
    import ml_dtypes
    f8 = ml_dtypes.float8_e4m3
    bf = ml_dtypes.bfloat16

    q = np.asarray(inputs["q"], dtype=np.float32)
    k = np.asarray(inputs["k"], dtype=np.float32)
    v = np.asarray(inputs["v"], dtype=np.float32)
    wq = np.asarray(inputs["wq"], dtype=np.float32)
    wk = np.asarray(inputs["wk"], dtype=np.float32)
    wv_ = np.asarray(inputs["wv"], dtype=np.float32)
    wo = np.asarray(inputs["wo"], dtype=np.float32)
    bq = np.asarray(inputs["bq"], dtype=np.float32)
    bk = np.asarray(inputs["bk"], dtype=np.float32)
    bv_ = np.asarray(inputs["bv"], dtype=np.float32)
    bo = np.asarray(inputs["bo"], dtype=np.float32)
    temp = np.asarray(inputs["temperature"], dtype=np.float32).reshape(HEADS)

    wb = np.zeros((C1, 900), dtype=np.float32)
    wb[0:128, 772:900] = np.eye(128, dtype=np.float32)
    wb[0, 0:192] = bq
    wb[1:, 0:192] = wq.T
    wb[0, 192:384] = bk
    wb[1:, 192:384] = wk.T
    wb[0:192, 384:576] = wv_
    wb[0:192, 576:768] = wo.T
    wb[0:192, 768] = bv_
    wf = np.zeros((1, 200), dtype=np.float32)
    wf[0, 0:192] = bo
    tcol = np.repeat(temp, CH)                  # (192,) per-channel temp
    wtc = np.zeros((128, 4), dtype=np.float32)
    wtc[:, 0] = 1.0 / np.maximum(tcol[0:128] ** 2, 1e-30)
    wtc[:, 1] = np.sign(tcol[0:128]) + (tcol[0:128] == 0)
    wtc[0:64, 2] = 1.0 / np.maximum(tcol[128:192] ** 2, 1e-30)
    wtc[0:64, 3] = np.sign(tcol[128:192]) + (tcol[128:192] == 0)

    shared = {
        "wb": np.ascontiguousarray(wb.astype(bf)),
        "wf": wf,
        "wtc": wtc,
    }
    ones_col = np.ones((HW, 1), dtype=np.float32)
    in_maps = []
    for b in range(B):
        m = dict(shared)
        aq = np.zeros((HW, C1P), dtype=np.float32)
        aq[:, 0:1] = 1.0
        aq[:, 1:C1] = q[b].reshape(DIM, HW).T
        ak = np.zeros((HW, C1P), dtype=np.float32)
        ak[:, 0:1] = 1.0
        ak[:, 1:C1] = k[b].reshape(DIM, HW).T
        m["q8"] = np.ascontiguousarray(
            aq.astype(f8).reshape(HW // 16, W1))
        m["k8"] = np.ascontiguousarray(
            ak.astype(f8).reshape(HW // 16, W1))
        # permute pixels [even|odd] per 256-block, append ones row
        vp = v[b].reshape(DIM, NG2, 128, 2).transpose(0, 1, 3, 2)
        vb = np.concatenate(
            [vp.reshape(DIM, HW), np.ones((1, HW), np.float32)], axis=0)
        m["vb"] = np.ascontiguousarray(vb.astype(bf))
        in_maps.append(m)
    return in_maps


def _get_runner():
    """Compile once and cache a sharded-jit runner."""
    if "runner" in _CACHE:
        return _CACHE["runner"]
    import jax
    import jax.numpy as jnp
    from jax.sharding import Mesh, PartitionSpec
    from jax.experimental.shard_map import shard_map
    from concourse import bass2jax, mybir as mb
    from concourse.bass2jax import _bass_exec_p, partition_id_tensor

    bass2jax.install_neuronx_cc_hook()
    nc = _get_nc()

    partition_name = nc.partition_id_tensor.name if nc.partition_id_tensor else None
    in_names, out_names, out_avals = [], [], []
    for alloc in nc.m.functions[0].allocations:
        if not isinstance(alloc, mb.MemoryLocationSet):
            continue
        name = alloc.memorylocations[0].name
        if alloc.kind == "ExternalInput":
            if name != partition_name:
                in_names.append(name)
        elif alloc.kind == "ExternalOutput":
            out_names.append(name)
            out_avals.append(jax.core.ShapedArray(
                tuple(alloc.tensor_shape), mb.dt.np(alloc.dtype)))
    n_params = len(in_names)
    n_outs = len(out_avals)
    all_in_names = tuple(in_names + out_names +
                         ([partition_name] if partition_name else []))

    def _body(*args):
        operands = list(args)
        if partition_name is not None:
            operands.append(partition_id_tensor())
        return tuple(_bass_exec_p.bind(
            *operands,
            out_avals=tuple(out_avals),
            in_names=all_in_names,
            out_names=tuple(out_names),
            lowering_input_output_aliases=(),
            sim_require_finite=True,
            sim_require_nnan=True,
            nc=nc,
        ))

    devices = jax.devices()[:B]
    mesh = Mesh(np.asarray(devices), ("core",))
    in_specs = (PartitionSpec("core"),) * (n_params + n_outs)
    out_specs = (PartitionSpec("core"),) * n_outs
    donate = tuple(range(n_params, n_params + n_outs))
    sharded = jax.jit(
        shard_map(_body, mesh=mesh, in_specs=in_specs, out_specs=out_specs,
                  check_rep=False),
        donate_argnums=donate, keep_unused=True)

    zero_shapes = [(B * a.shape[0], *a.shape[1:]) for a in out_avals]
    zero_dtypes = [a.dtype for a in out_avals]

    def run(in_maps):
        concat_in = [
            np.concatenate([np.asarray(in_maps[c][nm]) for c in range(B)], axis=0)
            for nm in in_names
        ]
        zeros = [jnp.zeros(s, d) for s, d in zip(zero_shapes, zero_dtypes)]
        outs = sharded(*concat_in, *zeros)
        return {
            nm: np.asarray(outs[i]).reshape(B, *out_avals[i].shape)
            for i, nm in enumerate(out_names)
        }

    _CACHE["runner"] = run
    return run


def _prebuild():
    """Compile the NEFF and warm the jit at import time; never break import.
    Uses non-degenerate dummy data so norms stay positive (rsqrt-safe)."""
    try:
        import ml_dtypes
        run = _get_runner()
        zq = np.ones((HW // 16, W1), dtype=ml_dtypes.float8_e4m3)
        zv = np.ones((C1, HW), dtype=ml_dtypes.bfloat16)
        zw = np.full((C1, 900), 0.01, dtype=ml_dtypes.bfloat16)
        zf = np.full((1, 200), 0.01, dtype=np.float32)
        zt = np.ones((128, 4), dtype=np.float32)
        run([{"q8": zq, "k8": zq, "vb": zv, "wb": zw, "wf": zf, "wtc": zt}
             for _ in range(B)])
    except Exception:
        _CACHE.clear()


def kernel(q, k, v, wq, bq, wk, bk, wv, bv, wo, bo, temperature):
    run = _get_runner()
    in_maps = _make_in_maps(dict(q=q, k=k, v=v, wq=wq, bq=bq, wk=wk, bk=bk,
                                 wv=wv, bv=bv, wo=wo, bo=bo,
                                 temperature=temperature))
    out = run(in_maps)["out"]  # (B, NST, GPS, 128, 384) bf16
    out = out.astype(np.float32).reshape(B, HW, DIM)
    out = np.ascontiguousarray(out.transpose(0, 2, 1)).reshape(B, DIM, 128, 128)
    return out


import os as _os
if not _os.environ.get("KERNEL_NO_PREBUILD"):
    _prebuild()
